# revision 35
# baseline (speedup 1.0000x reference)
"""Sharded causal-attention kernel for 8 trn2 NeuronCores.

DP over batch (2) x TP over head groups (4 heads/core). Each core: qkv projection
(its heads) + RoPE + causal SDPA (scores kept transposed; softmax denominator via a
ones-column in the PV matmul) + its 256-row slice of the o_proj contraction, returning
a transposed partial [HID, S]; the host sums 4 partials per batch. bf16 matmuls,
fp32 PSUM accumulation.

v2: consumption-ordered fine-grained input DMA across the sync/scalar/gpsimd
DGE queues (first matmul gates on ~256KB instead of 1MB, deferred weight loads
slotted behind compute-gated ops); causal-live-column score matmuls; PV delayed
one k-tile behind exp so the PE has independent work while ACT computes exp;
softmax division with fast reciprocal + GpSimd broadcasts hoisted ahead of the
numerator path and small copies split across ACT/DVE; RoPE PSUM->SBUF copy and
rotate matmul split by column halves across ACT/DVE; final o_proj output DMA
fanned out per-128KB tile across queues with copies split across both engines.
"""

import sys

sys.path.insert(0, "/opt/trn_rl_repo")

from contextlib import ExitStack

import numpy as np
import ml_dtypes

import concourse.bass as bass
import concourse.mybir as mybir
import concourse.tile as tile
from concourse import bacc

FP = mybir.dt.float32
BF = mybir.dt.bfloat16
EXP = mybir.ActivationFunctionType.Exp

B, S, HID = 2, 2048, 1024
H, D = 16, 64
QC = 512
KT = 128
NQC = S // QC
NKT = S // KT
KHID = HID // 128


def build_program(debug_outputs=False):
    nc = bacc.Bacc("TRN2", target_bir_lowering=False, debug=False, num_devices=8, num_swdge_queues=4)

    hsT = nc.dram_tensor("hsT", [128, NQC * KHID * QC], BF, kind="ExternalInput").ap()
    wqkT = nc.dram_tensor("wqkT", [128, KHID * 512], BF, kind="ExternalInput").ap()
    wvT = nc.dram_tensor("wvT", [128, KHID * 256], BF, kind="ExternalInput").ap()
    woT = nc.dram_tensor("woT", [128, 2 * HID], BF, kind="ExternalInput").ap()
    cos2T = nc.dram_tensor("cos2T", [64, S], BF, kind="ExternalInput").ap()
    ssin2T = nc.dram_tensor("ssin2T", [64, S], BF, kind="ExternalInput").ap()
    maskD = nc.dram_tensor("maskD", [128, 256], BF, kind="ExternalInput").ap()
    pmat = nc.dram_tensor("pmat", [128, 128], BF, kind="ExternalInput").ap()
    outT = nc.dram_tensor("outT", [128, NQC * 8 * QC], BF, kind="ExternalOutput").ap()
    dbg = None
    if debug_outputs:
        dbg = {
            "dbg_qk": nc.dram_tensor("dbg_qk", [512, S], BF, kind="ExternalOutput").ap(),
            "dbg_v": nc.dram_tensor("dbg_v", [128, NKT * 4 * 65], BF, kind="ExternalOutput").ap(),
            "dbg_att": nc.dram_tensor("dbg_att", [256, S], BF, kind="ExternalOutput").ap(),
        }

    with tile.TileContext(nc) as tc:
        build_tile_program(tc, hsT, wqkT, wvT, woT, cos2T, ssin2T, maskD, pmat, outT, dbg)
    nc.compile()
    return nc


def build_tile_program(tc, hsT, wqkT, wvT, woT, cos2T, ssin2T, maskD, pmat, outT, dbg=None):
    nc = tc.nc
    with ExitStack() as ctx:
        const = ctx.enter_context(tc.tile_pool(name="const", bufs=1))
        persist = ctx.enter_context(tc.tile_pool(name="persist", bufs=1))
        work = ctx.enter_context(tc.tile_pool(name="work", bufs=5))
        posbp = ctx.enter_context(tc.tile_pool(name="posbp", bufs=12))
        expp = ctx.enter_context(tc.tile_pool(name="expp", bufs=8))
        small = ctx.enter_context(tc.tile_pool(name="small", bufs=4))
        ps_main = ctx.enter_context(tc.tile_pool(name="ps_main", bufs=4, space="PSUM"))
        ps_sc = ctx.enter_context(tc.tile_pool(name="ps_sc", bufs=2, space="PSUM"))

        # ---- critical first wave: exactly what the first proj matmuls gate on,
        # in consumption order, spread across all five engine DMA queues so no
        # single queue serializes the gate ----
        wqk_sb = const.tile([128, KHID, 512], BF, name="wqk_sb")
        hs_sb = const.tile([128, NQC, KHID, QC], BF, name="hs_sb")
        cos_sb = const.tile([128, S], BF, name="cos_sb")
        ssin_sb = const.tile([128, S], BF, name="ssin_sb")
        pmat_sb = const.tile([128, 128], BF, name="pmat_sb")
        nc.sync.dma_start(
            wqk_sb[:, 0:2, :],
            wqkT[:, 0:1024].rearrange("p (k m) -> p k m", k=2),
        )
        nc.gpsimd.dma_start(hs_sb[:, 0, 0:1, :], hsT[:, 0:QC].rearrange("p (k s) -> p k s", k=1))
        nc.scalar.dma_start(cos_sb[0:64, 0:QC], cos2T[:, 0:QC])
        nc.sync.dma_start(pmat_sb[:], pmat[:])
        nc.scalar.dma_start(ssin_sb[0:64, 0:QC], ssin2T[:, 0:QC])
        for j, eng in ((1, nc.scalar), (2, nc.sync), (3, nc.scalar)):
            eng.dma_start(
                wqk_sb[:, 2 * j:2 * j + 2, :],
                wqkT[:, j * 1024:(j + 1) * 1024].rearrange("p (k m) -> p k m", k=2),
            )
        for lo_, hi_ in ((1, 3), (3, 5), (5, 8)):
            nc.gpsimd.dma_start(
                hs_sb[:, 0, lo_:hi_, :],
                hsT[:, lo_ * QC:hi_ * QC].rearrange("p (k s) -> p k s", k=hi_ - lo_),
            )
        nc.vector.tensor_copy(cos_sb[64:128, 0:QC], cos_sb[0:64, 0:QC])
        nc.vector.tensor_copy(ssin_sb[64:128, 0:QC], ssin_sb[0:64, 0:QC])
        tri_sb = const.tile([128, 2, 128], BF, name="tri_sb")
        # remaining hs chunks: gpsimd queue, issued behind the chunk-0 pieces
        for t in range(1, NQC):
            for h in range(2):
                nc.gpsimd.dma_start(
                    hs_sb[:, t, 4 * h:4 * h + 4, :],
                    hsT[:, (t * KHID + 4 * h) * QC:(t * KHID + 4 * h + 4) * QC].rearrange(
                        "p (k s) -> p k s", k=4),
                )
        wv_sb = const.tile([128, KHID, 256], BF, name="wv_sb")
        wo_sb = const.tile([128, 2, HID], BF, name="wo_sb")

        qkT = persist.tile([128, 4, S], BF, name="qkT")
        l_tiles = persist.tile([64, 8, QC], FP, name="l_tiles")
        v_sb = persist.tile([128, NKT, 4 * 65], BF, name="v_sb2")
        att_sb = persist.tile([128, 2, S], BF, name="att_sb2")
        nc.vector.memset(
            v_sb.rearrange("p t (h c) -> p t h c", c=65)[:, :, :, 64:65], 1.0
        )

        def proj_chunk(rb, t):
            csl = slice(t * QC, (t + 1) * QC)
            ps = ps_main.tile([128, QC], FP, name="ps_qk", tag="ps")
            for kk in range(KHID):
                nc.tensor.matmul(
                    ps[:],
                    wqk_sb[:, kk, rb * 128:(rb + 1) * 128],
                    hs_sb[:, t, kk, :],
                    start=(kk == 0),
                    stop=(kk == KHID - 1),
                )
            x = work.tile([128, QC], BF, name="x_rope", tag="xrope")
            half = QC // 2
            nc.scalar.copy(x[:, 0:half], ps[:, 0:half])
            nc.vector.tensor_copy(x[:, half:QC], ps[:, half:QC])
            # signed rotate-half on the PE: xs = P @ x (P carries the +-1),
            # split by column half so each starts as soon as its copy lands
            xs_ps = ps_main.tile([128, QC], FP, name="xs_ps", tag="ps")
            nc.tensor.matmul(xs_ps[:, 0:half], pmat_sb[:], x[:, 0:half], start=True, stop=True)
            nc.tensor.matmul(xs_ps[:, half:QC], pmat_sb[:], x[:, half:QC], start=True, stop=True)
            t1 = work.tile([128, QC], BF, name="t1_rope", tag="t1rope")
            t2 = work.tile([128, QC], BF, name="t2_rope", tag="t2rope")
            nc.vector.tensor_mul(t1[:], x[:], cos_sb[:, csl])
            nc.vector.tensor_mul(t2[:], xs_ps[:], ssin_sb[:, csl])
            nc.vector.tensor_add(qkT[:, rb, csl], t1[:], t2[:])

        def v_proj(tt):
            psv = ps_main.tile([128, 256], FP, name="ps_v", tag="ps")
            for kk in range(KHID):
                nc.tensor.matmul(
                    psv[:],
                    hs_sb[:, tt // 4, kk, (tt % 4) * 128:(tt % 4 + 1) * 128],
                    wv_sb[:, kk, :],
                    start=(kk == 0),
                    stop=(kk == KHID - 1),
                )
            nc.scalar.copy(
                v_sb[:, tt, :].rearrange("p (h c) -> p h c", c=65)[:, :, 0:64],
                psv[:].rearrange("p (h c) -> p h c", c=64),
            )

        def attention_unit(qi, l0, l1):
            """Fused scores^T -> exp -> PV for all four heads at q-chunk qi.

            Both head-pairs interleave per k-tile so each exp has ~2x the PE
            cover, and the PV batch trails one k-tile behind. The four PV
            accumulators occupy all four ps_main buffers for the unit.
            Denominators land in l0/l1 rows 0 and 32."""
            qsl = slice(qi * QC, (qi + 1) * QC)
            nki = 4 * qi + 4
            po = [ps_main.tile([65, QC], FP, name=f"po{h}", tag="ps") for h in range(4)]

            def emit_pv(ki, e_a, e_b, lo):
                for h, (e_, sub) in enumerate(((e_a, 0), (e_a, 1), (e_b, 0), (e_b, 1))):
                    nc.tensor.matmul(
                        po[h][:, lo:QC], v_sb[:, ki, h * 65:(h + 1) * 65],
                        e_[:, sub, lo:QC],
                        start=(ki == 0), stop=(ki == nki - 1),
                    )

            pending = []
            for ki in range(nki):
                ksl = slice(ki * KT, (ki + 1) * KT)
                j = ki - 4 * qi
                lo = 0 if j < 0 else 128 * j  # first live q column in this chunk
                es = []
                for pair in range(2):
                    psc = ps_sc.tile([128, 2, QC], FP, name="psc", tag="sc")
                    nc.tensor.matmul(
                        psc[:, 0, lo:QC], qkT[0:64, 2 + pair, ksl],
                        qkT[0:64, pair, qi * QC + lo:(qi + 1) * QC],
                        start=True, stop=True,
                    )
                    nc.tensor.matmul(
                        psc[:, 1, lo:QC], qkT[64:128, 2 + pair, ksl],
                        qkT[64:128, pair, qi * QC + lo:(qi + 1) * QC],
                        start=True, stop=True,
                    )
                    e = expp.tile([128, 2, QC], BF, name="e", tag="exp")
                    nc.scalar.activation(
                        e[:, :, lo:QC], psc[:, :, lo:QC], EXP, scale=0.125
                    )
                    if j >= 0:
                        nc.vector.tensor_mul(
                            e[:, :, lo:lo + 128], e[:, :, lo:lo + 128], tri_sb[:]
                        )
                    es.append(e)
                if pending:
                    emit_pv(*pending.pop(0))
                pending.append((ki, es[0], es[1], lo))
            for p_ in pending:
                emit_pv(*p_)
            # epilogue: denominator rows to l0/l1, numerators to SBUF bf16
            nc.vector.tensor_copy(l0[0:1, :], po[0][64:65, :])
            nc.scalar.copy(l0[32:33, :], po[1][64:65, :])
            nc.vector.tensor_copy(l1[0:1, :], po[2][64:65, :])
            nc.scalar.copy(l1[32:33, :], po[3][64:65, :])
            po_sb = []
            for h in range(4):
                t_ = posbp.tile([64, QC], BF, name=f"po_sb{h}", tag="posb")
                if h % 2 == 0:
                    nc.vector.tensor_copy(t_[:], po[h][0:64, :])
                else:
                    nc.scalar.copy(t_[:], po[h][0:64, :])
                po_sb.append(t_)
            return (po_sb[0], po_sb[1]), (po_sb[2], po_sb[3])

        def division_pre(l_pair):
            """1/l for both heads of a pair, broadcast to 64 partitions.

            Only depends on the denominator rows, so the gpsimd broadcasts
            overlap the next attention unit / o_proj on the PE."""
            rl = small.tile([64, QC], FP, name="rl", tag="rl")
            nc.vector.reciprocal_approx_fast(out=rl[:], in_=l_pair[:])
            rb0_ = small.tile([64, QC], FP, name="rb0_", tag="rbb", bufs=6)
            nc.gpsimd.partition_broadcast(rb0_[:], rl[0:1, :])
            rlrow = small.tile([1, QC], FP, name="rlrow", tag="rlrow", bufs=4)
            nc.scalar.copy(rlrow[:], rl[32:33, :])
            rb1_ = small.tile([64, QC], FP, name="rb1_", tag="rbb", bufs=6)
            nc.gpsimd.partition_broadcast(rb1_[:], rlrow[:])
            return rb0_, rb1_

        def division_post(pair, qi, rb, po_sb0, po_sb1):
            qsl = slice(qi * QC, (qi + 1) * QC)
            rb0_, rb1_ = rb
            nc.vector.tensor_mul(att_sb[0:64, pair, qsl], po_sb0[0:64, :], rb0_[:])
            nc.vector.tensor_mul(att_sb[64:128, pair, qsl], po_sb1[0:64, :], rb1_[:])

        def oproj(qi, last=False, tailish=False):
            qsl = slice(qi * QC, (qi + 1) * QC)
            out_engs = (nc.gpsimd, nc.sync, nc.scalar, nc.gpsimd)
            if last:
                for half in range(2):
                    ow = work.tile([128, 4, QC], BF, name="ow", tag="ow")
                    for oi in range(4):
                        ot = half * 4 + oi
                        pw = ps_main.tile([128, QC], FP, name="pw", tag="ps")
                        for p in range(2):
                            nc.tensor.matmul(
                                pw[:],
                                wo_sb[:, p, ot * 128:(ot + 1) * 128],
                                att_sb[:, p, qsl],
                                start=(p == 0),
                                stop=(p == 1),
                            )
                        nc.scalar.copy(ow[:, oi, 0:QC // 2], pw[:, 0:QC // 2])
                        nc.vector.tensor_copy(ow[:, oi, QC // 2:QC], pw[:, QC // 2:QC])
                        off = (qi * 8 + half * 4 + oi) * QC
                        out_engs[oi].dma_start(outT[:, off:off + QC], ow[:, oi, :])
                return
            for half in range(2):
                ow = work.tile([128, 4, QC], BF, name="ow", tag="ow")
                for oi in range(4):
                    ot = half * 4 + oi
                    pw = ps_main.tile([128, QC], FP, name="pw", tag="ps")
                    for p in range(2):
                        nc.tensor.matmul(
                            pw[:],
                            wo_sb[:, p, ot * 128:(ot + 1) * 128],
                            att_sb[:, p, qsl],
                            start=(p == 0),
                            stop=(p == 1),
                        )
                    if (oi % 2 == 1) if tailish else (ot % 2 == 1):
                        nc.scalar.copy(ow[:, oi, :], pw[:])
                    else:
                        nc.vector.tensor_copy(ow[:, oi, :], pw[:])
                off = (qi * 2 + half) * 4 * QC
                (nc.gpsimd if half == 0 else nc.sync).dma_start(
                    outT[:, off:off + 4 * QC].rearrange("p (o s) -> p o s", o=4),
                    ow[:],
                )

        # emission: pair0 projections up front with the deferred weight loads
        # slotted behind compute-gated ops so they don't steal SDMA bandwidth
        # from the chunk-0 / wqk gates; then pair1 projections, v, attention
        # and (one chunk behind) o_proj interleaved per q chunk.
        for t in range(NQC):
            if t == 1:
                nc.vector.tensor_copy(cos_sb[64:128, QC:S], cos_sb[0:64, QC:S])
                nc.vector.tensor_copy(ssin_sb[64:128, QC:S], ssin_sb[0:64, QC:S])
            proj_chunk(0, t)
            if t == 0:
                nc.scalar.dma_start(cos_sb[0:64, QC:S], cos2T[:, QC:S])
                nc.scalar.dma_start(ssin_sb[0:64, QC:S], ssin2T[:, QC:S])
                nc.scalar.dma_start(tri_sb[:], maskD.rearrange("p (r c) -> p r c", r=2))
            proj_chunk(2, t)
            if t == 0:
                for h in range(2):
                    nc.scalar.dma_start(
                        wv_sb[:, 4 * h:4 * h + 4, :],
                        wvT[:, h * 1024:(h + 1) * 1024].rearrange("p (k m) -> p k m", k=4),
                    )
            if t == 1:
                for h in range(2):
                    nc.scalar.dma_start(wo_sb[:, h, :], woT[:, h * HID:(h + 1) * HID])
        nc.vector.memset(l_tiles[:], 1.0)
        qi_order = [1, 2, 3, 0]
        loaded = 0
        prev = None
        for qi in qi_order:
            while loaded <= min(qi + 1, NQC - 1):
                proj_chunk(1, loaded)
                proj_chunk(3, loaded)
                for tt in range(4 * loaded, 4 * loaded + 4):
                    v_proj(tt)
                loaded += 1
            l0 = l_tiles[:, 2 * qi, :]
            l1 = l_tiles[:, 2 * qi + 1, :]
            pa, pb = attention_unit(qi, l0, l1)
            rlb0 = division_pre(l0)
            rlb1 = division_pre(l1)
            if prev is not None:
                oproj(prev, tailish=(qi == qi_order[-1]))
            division_post(0, qi, rlb0, *pa)
            division_post(1, qi, rlb1, *pb)
            prev = qi
        oproj(prev, last=True)

        if dbg is not None:
            for rb in range(4):
                nc.sync.dma_start(dbg["dbg_qk"][rb * 128:(rb + 1) * 128, :], qkT[:, rb, :])
            nc.sync.dma_start(dbg["dbg_v"][:], v_sb.rearrange("p t c -> p (t c)"))
            for p in range(2):
                nc.sync.dma_start(dbg["dbg_att"][p * 128:(p + 1) * 128, :], att_sb[:, p, :])


# ---------- host-side shard preparation ----------

def make_core_inputs(hidden_states, cos, sin, w_qkv, w_o):
    """Returns list of 8 in_maps (numpy, bf16 where needed)."""
    bf = ml_dtypes.bfloat16
    hs = np.asarray(hidden_states, np.float32)
    cos = np.asarray(cos, np.float32)
    sin = np.asarray(sin, np.float32)
    w_qkv = np.asarray(w_qkv, np.float32)
    w_o = np.asarray(w_o, np.float32)

    cosT = cos.T
    sinT = sin.T
    cos2T = np.ascontiguousarray(cosT).astype(bf)
    ssin2T = np.ascontiguousarray(sinT).astype(bf)
    # signed rotate-half permutation: out[m] = sign(m) * x[partner(m)]
    # lhsT layout: pmat[k, m] = sign(m) at k = partner(m)
    pmat = np.zeros((128, 128), np.float32)
    for m in range(128):
        d = m % 64
        base = m - d
        if d < 32:
            pmat[base + d + 32, m] = -1.0
        else:
            pmat[base + d - 32, m] = 1.0
    pmat = pmat.astype(bf)

    kp = np.arange(128)[:, None]
    cc = np.arange(128)[None, :]
    tri = (kp <= cc).astype(bf)
    maskD = np.concatenate([tri, tri], axis=1)

    def swz(a):
        # [K*128, M] -> [128, K*M] partition-major (matches SBUF tiles)
        k = a.shape[0] // 128
        return np.ascontiguousarray(
            a.reshape(k, 128, a.shape[1]).transpose(1, 0, 2).reshape(128, -1))

    in_maps = []
    for c in range(8):
        b, g = divmod(c, 4)
        heads = range(4 * g, 4 * g + 4)
        hsT = np.ascontiguousarray(hs[b].T).astype(bf)  # [HID, S]
        # chunk-major swizzle: [128, t, kk, s']
        hs2 = np.ascontiguousarray(
            hsT.reshape(8, 128, 4, 512).transpose(1, 2, 0, 3).reshape(128, -1))
        wq = np.concatenate([w_qkv[h * 64:(h + 1) * 64] for h in heads], 0)
        wk = np.concatenate([w_qkv[HID + h * 64:HID + (h + 1) * 64] for h in heads], 0)
        wv = np.concatenate([w_qkv[2 * HID + h * 64:2 * HID + (h + 1) * 64] for h in heads], 0)
        wqkT = swz(np.ascontiguousarray(np.concatenate([wq, wk], 0).T).astype(bf))
        wvT = swz(np.ascontiguousarray(wv.T).astype(bf))
        woT = swz(np.ascontiguousarray(
            np.concatenate([w_o[:, h * 64:(h + 1) * 64] for h in heads], 1).T
        ).astype(bf))
        in_maps.append({
            "hsT": hs2, "wqkT": wqkT, "wvT": wvT, "woT": woT,
            "cos2T": cos2T, "ssin2T": ssin2T, "maskD": maskD, "pmat": pmat,
        })
    return in_maps


def unswizzle_out(o2):
    # [128, qi*half*oi*512] -> outT [1024, 2048]
    a = o2.reshape(128, NQC, 2, 4, QC)
    return np.ascontiguousarray(
        a.transpose(2, 3, 0, 1, 4).reshape(HID, S))


def unshard(outTs):
    out = np.zeros((B, S, HID), np.float32)
    for c, o2 in enumerate(outTs):
        out[c // 4] += unswizzle_out(o2).T.astype(np.float32)
    return out


# ---------- standalone kernel entry ----------

from concourse.bass_utils import run_bass_kernel_spmd

_CACHED_NC = None


def get_program():
    global _CACHED_NC
    if _CACHED_NC is None:
        _CACHED_NC = build_program()
    return _CACHED_NC


def run(inputs, trace=False):
    nc = get_program()
    in_maps = make_core_inputs(**inputs)
    res = run_bass_kernel_spmd(nc, in_maps, core_ids=list(range(8)), trace=trace)
    out = np.zeros((B, S, HID), np.float32)
    for c, r in enumerate(res.results):
        out[c // 4] += unswizzle_out(r["outT"]).T.astype(np.float32)
    return out, res


def kernel(**inputs):
    out, _ = run(inputs, trace=False)
    return out


# revision 36
# speedup vs baseline: 1.1649x; 1.1649x over previous
"""Sharded causal-attention kernel for 8 trn2 NeuronCores.

DP over batch (2) x TP over head groups (4 heads/core). Each core: qkv projection
(its heads) + RoPE + causal SDPA (scores kept transposed; softmax denominator via a
ones-column in the PV matmul) + its 256-row slice of the o_proj contraction, returning
a transposed partial [HID, S]; the host sums 4 partials per batch. bf16 matmuls,
fp32 PSUM accumulation.

v2: consumption-ordered fine-grained input DMA across the sync/scalar/gpsimd
DGE queues (first matmul gates on ~256KB instead of 1MB, deferred weight loads
slotted behind compute-gated ops); causal-live-column score matmuls; PV delayed
one k-tile behind exp so the PE has independent work while ACT computes exp;
softmax division with fast reciprocal + GpSimd broadcasts hoisted ahead of the
numerator path and small copies split across ACT/DVE; RoPE PSUM->SBUF copy and
rotate matmul split by column halves across ACT/DVE; final o_proj output DMA
fanned out per-128KB tile across queues with copies split across both engines.
"""

import sys

sys.path.insert(0, "/opt/trn_rl_repo")

from contextlib import ExitStack

import numpy as np
import ml_dtypes

import concourse.bass as bass
import concourse.mybir as mybir
import concourse.tile as tile
from concourse import bacc

FP = mybir.dt.float32
BF = mybir.dt.bfloat16
EXP = mybir.ActivationFunctionType.Exp

B, S, HID = 2, 2048, 1024
H, D = 16, 64
QC = 512
KT = 128
NQC = S // QC
NKT = S // KT
KHID = HID // 128


def build_program(debug_outputs=False):
    nc = bacc.Bacc("TRN2", target_bir_lowering=False, debug=False, num_devices=8, num_swdge_queues=4)

    hsT = nc.dram_tensor("hsT", [128, NQC * KHID * QC], BF, kind="ExternalInput").ap()
    wqkT = nc.dram_tensor("wqkT", [128, KHID * 512], BF, kind="ExternalInput").ap()
    wvT = nc.dram_tensor("wvT", [128, KHID * 256], BF, kind="ExternalInput").ap()
    woT = nc.dram_tensor("woT", [128, 2 * HID], BF, kind="ExternalInput").ap()
    cos2T = nc.dram_tensor("cos2T", [64, S], BF, kind="ExternalInput").ap()
    ssin2T = nc.dram_tensor("ssin2T", [64, S], BF, kind="ExternalInput").ap()
    maskD = nc.dram_tensor("maskD", [128, 256], BF, kind="ExternalInput").ap()
    pmat = nc.dram_tensor("pmat", [128, 128], BF, kind="ExternalInput").ap()
    outT = nc.dram_tensor("outT", [128, NQC * 8 * QC], BF, kind="ExternalOutput").ap()
    dbg = None
    if debug_outputs:
        dbg = {
            "dbg_qk": nc.dram_tensor("dbg_qk", [512, S], BF, kind="ExternalOutput").ap(),
            "dbg_v": nc.dram_tensor("dbg_v", [128, NKT * 4 * 65], BF, kind="ExternalOutput").ap(),
            "dbg_att": nc.dram_tensor("dbg_att", [256, S], BF, kind="ExternalOutput").ap(),
        }

    with tile.TileContext(nc) as tc:
        build_tile_program(tc, hsT, wqkT, wvT, woT, cos2T, ssin2T, maskD, pmat, outT, dbg)
    nc.compile()
    return nc


def build_tile_program(tc, hsT, wqkT, wvT, woT, cos2T, ssin2T, maskD, pmat, outT, dbg=None):
    nc = tc.nc
    with ExitStack() as ctx:
        const = ctx.enter_context(tc.tile_pool(name="const", bufs=1))
        persist = ctx.enter_context(tc.tile_pool(name="persist", bufs=1))
        work = ctx.enter_context(tc.tile_pool(name="work", bufs=5))
        posbp = ctx.enter_context(tc.tile_pool(name="posbp", bufs=12))
        expp = ctx.enter_context(tc.tile_pool(name="expp", bufs=8))
        small = ctx.enter_context(tc.tile_pool(name="small", bufs=4))
        ps_main = ctx.enter_context(tc.tile_pool(name="ps_main", bufs=4, space="PSUM"))
        ps_sc = ctx.enter_context(tc.tile_pool(name="ps_sc", bufs=2, space="PSUM"))

        # ---- critical first wave: exactly what the first proj matmuls gate on,
        # in consumption order, spread across all five engine DMA queues so no
        # single queue serializes the gate ----
        wqk_sb = const.tile([128, KHID, 512], BF, name="wqk_sb")
        hs_sb = const.tile([128, NQC, KHID, QC], BF, name="hs_sb")
        cos_sb = const.tile([128, S], BF, name="cos_sb")
        ssin_sb = const.tile([128, S], BF, name="ssin_sb")
        pmat_sb = const.tile([128, 128], BF, name="pmat_sb")
        nc.sync.dma_start(
            wqk_sb[:, 0:2, :],
            wqkT[:, 0:1024].rearrange("p (k m) -> p k m", k=2),
        )
        nc.gpsimd.dma_start(hs_sb[:, 0, 0:1, :], hsT[:, 0:QC].rearrange("p (k s) -> p k s", k=1))
        nc.scalar.dma_start(cos_sb[0:64, 0:QC], cos2T[:, 0:QC])
        nc.sync.dma_start(pmat_sb[:], pmat[:])
        nc.scalar.dma_start(ssin_sb[0:64, 0:QC], ssin2T[:, 0:QC])
        for j, eng in ((1, nc.scalar), (2, nc.sync), (3, nc.scalar)):
            eng.dma_start(
                wqk_sb[:, 2 * j:2 * j + 2, :],
                wqkT[:, j * 1024:(j + 1) * 1024].rearrange("p (k m) -> p k m", k=2),
            )
        for lo_, hi_ in ((1, 3), (3, 5), (5, 8)):
            nc.gpsimd.dma_start(
                hs_sb[:, 0, lo_:hi_, :],
                hsT[:, lo_ * QC:hi_ * QC].rearrange("p (k s) -> p k s", k=hi_ - lo_),
            )
        nc.vector.tensor_copy(cos_sb[64:128, 0:QC], cos_sb[0:64, 0:QC])
        nc.vector.tensor_copy(ssin_sb[64:128, 0:QC], ssin_sb[0:64, 0:QC])
        tri_sb = const.tile([128, 2, 128], BF, name="tri_sb")
        # remaining hs chunks: gpsimd queue, issued behind the chunk-0 pieces
        for t in range(1, NQC):
            for h in range(2):
                nc.gpsimd.dma_start(
                    hs_sb[:, t, 4 * h:4 * h + 4, :],
                    hsT[:, (t * KHID + 4 * h) * QC:(t * KHID + 4 * h + 4) * QC].rearrange(
                        "p (k s) -> p k s", k=4),
                )
        wv_sb = const.tile([128, KHID, 256], BF, name="wv_sb")
        wo_sb = const.tile([128, 2, HID], BF, name="wo_sb")

        qkT = persist.tile([128, 4, S], BF, name="qkT")
        l_tiles = persist.tile([64, 8, QC], FP, name="l_tiles")
        v_sb = persist.tile([128, NKT, 4 * 65], BF, name="v_sb2")
        att_sb = persist.tile([128, 2, S], BF, name="att_sb2")
        nc.vector.memset(
            v_sb.rearrange("p t (h c) -> p t h c", c=65)[:, :, :, 64:65], 1.0
        )

        def proj_chunk(rb, t):
            csl = slice(t * QC, (t + 1) * QC)
            ps = ps_main.tile([128, QC], FP, name="ps_qk", tag="ps")
            for kk in range(KHID):
                nc.tensor.matmul(
                    ps[:],
                    wqk_sb[:, kk, rb * 128:(rb + 1) * 128],
                    hs_sb[:, t, kk, :],
                    start=(kk == 0),
                    stop=(kk == KHID - 1),
                )
            x = work.tile([128, QC], BF, name="x_rope", tag="xrope")
            half = QC // 2
            nc.scalar.copy(x[:, 0:half], ps[:, 0:half])
            nc.vector.tensor_copy(x[:, half:QC], ps[:, half:QC])
            # signed rotate-half on the PE: xs = P @ x (P carries the +-1),
            # split by column half so each starts as soon as its copy lands
            xs_ps = ps_main.tile([128, QC], FP, name="xs_ps", tag="ps")
            nc.tensor.matmul(xs_ps[:, 0:half], pmat_sb[:], x[:, 0:half], start=True, stop=True)
            nc.tensor.matmul(xs_ps[:, half:QC], pmat_sb[:], x[:, half:QC], start=True, stop=True)
            t1 = work.tile([128, QC], BF, name="t1_rope", tag="t1rope")
            t2 = work.tile([128, QC], BF, name="t2_rope", tag="t2rope")
            nc.vector.tensor_mul(t1[:], x[:], cos_sb[:, csl])
            nc.vector.tensor_mul(t2[:], xs_ps[:], ssin_sb[:, csl])
            nc.vector.tensor_add(qkT[:, rb, csl], t1[:], t2[:])

        def v_proj(tt):
            psv = ps_main.tile([128, 256], FP, name="ps_v", tag="ps")
            for kk in range(KHID):
                nc.tensor.matmul(
                    psv[:],
                    hs_sb[:, tt // 4, kk, (tt % 4) * 128:(tt % 4 + 1) * 128],
                    wv_sb[:, kk, :],
                    start=(kk == 0),
                    stop=(kk == KHID - 1),
                )
            nc.scalar.copy(
                v_sb[:, tt, :].rearrange("p (h c) -> p h c", c=65)[:, :, 0:64],
                psv[:].rearrange("p (h c) -> p h c", c=64),
            )

        def attention_unit(pair, qi, l_pair, copy_out=True):
            """scores^T -> exp -> PV for heads (2*pair, 2*pair+1).

            PV for k-tile ki is emitted after the psc matmuls of ki+1 so the PE
            has independent work while ACT computes exp(ki). Returns the
            numerators: SBUF bf16 copies if copy_out, else the PSUM tiles
            (pair0's division runs soon enough to read PSUM directly).
            Denominators land in l_pair rows 0 and 32."""
            qsl = slice(qi * QC, (qi + 1) * QC)
            nki = 4 * qi + 4
            po0 = ps_main.tile([65, QC], FP, name="po0", tag="ps")
            po1 = ps_main.tile([65, QC], FP, name="po1", tag="ps")
            h0 = 2 * pair
            h1 = 2 * pair + 1

            def emit_pv(ki, e, lo):
                nc.tensor.matmul(
                    po0[:, lo:QC], v_sb[:, ki, h0 * 65:(h0 + 1) * 65], e[:, 0, lo:QC],
                    start=(ki == 0), stop=(ki == nki - 1),
                )
                nc.tensor.matmul(
                    po1[:, lo:QC], v_sb[:, ki, h1 * 65:(h1 + 1) * 65], e[:, 1, lo:QC],
                    start=(ki == 0), stop=(ki == nki - 1),
                )

            pending = []
            for ki in range(nki):
                ksl = slice(ki * KT, (ki + 1) * KT)
                j = ki - 4 * qi
                lo = 0 if j < 0 else 128 * j  # first live q column in this chunk
                psc = ps_sc.tile([128, 2, QC], FP, name="psc", tag="sc")
                nc.tensor.matmul(
                    psc[:, 0, lo:QC], qkT[0:64, 2 + pair, ksl],
                    qkT[0:64, pair, qi * QC + lo:(qi + 1) * QC],
                    start=True, stop=True,
                )
                nc.tensor.matmul(
                    psc[:, 1, lo:QC], qkT[64:128, 2 + pair, ksl],
                    qkT[64:128, pair, qi * QC + lo:(qi + 1) * QC],
                    start=True, stop=True,
                )
                e = expp.tile([128, 2, QC], BF, name="e", tag="exp")
                nc.scalar.activation(
                    e[:, :, lo:QC], psc[:, :, lo:QC], EXP, scale=0.125
                )
                if j >= 0:
                    nc.vector.tensor_mul(
                        e[:, :, lo:lo + 128], e[:, :, lo:lo + 128], tri_sb[:]
                    )
                if len(pending) >= 1:
                    emit_pv(*pending.pop(0))
                pending.append((ki, e, lo))
            for p_ in pending:
                emit_pv(*p_)
            nc.scalar.copy(l_pair[0:1, :], po0[64:65, :])
            nc.scalar.copy(l_pair[32:33, :], po1[64:65, :])
            if not copy_out:
                return po0, po1
            # free PSUM fast: numerators to SBUF bf16
            po_sb0 = posbp.tile([64, QC], BF, name="po_sb0", tag="posb")
            po_sb1 = posbp.tile([64, QC], BF, name="po_sb1", tag="posb")
            nc.vector.tensor_copy(po_sb0[:], po0[0:64, :])
            nc.scalar.copy(po_sb1[:], po1[0:64, :])
            return po_sb0, po_sb1

        def division_pre(l_pair):
            """1/l for both heads of a pair, broadcast to 64 partitions.

            Only depends on the denominator rows, so the gpsimd broadcasts
            overlap the next attention unit / o_proj on the PE."""
            rl = small.tile([64, QC], FP, name="rl", tag="rl")
            nc.vector.reciprocal_approx_fast(out=rl[:], in_=l_pair[:])
            rb0_ = small.tile([64, QC], FP, name="rb0_", tag="rbb", bufs=6)
            nc.gpsimd.partition_broadcast(rb0_[:], rl[0:1, :])
            rlrow = small.tile([1, QC], FP, name="rlrow", tag="rlrow", bufs=4)
            nc.scalar.copy(rlrow[:], rl[32:33, :])
            rb1_ = small.tile([64, QC], FP, name="rb1_", tag="rbb", bufs=6)
            nc.gpsimd.partition_broadcast(rb1_[:], rlrow[:])
            return rb0_, rb1_

        def division_post(pair, qi, rb, po_sb0, po_sb1):
            qsl = slice(qi * QC, (qi + 1) * QC)
            rb0_, rb1_ = rb
            nc.vector.tensor_mul(att_sb[0:64, pair, qsl], po_sb0[0:64, :], rb0_[:])
            nc.vector.tensor_mul(att_sb[64:128, pair, qsl], po_sb1[0:64, :], rb1_[:])

        def oproj(qi, last=False, tailish=False):
            qsl = slice(qi * QC, (qi + 1) * QC)
            out_engs = (nc.gpsimd, nc.sync, nc.scalar, nc.gpsimd)
            if last:
                for half in range(2):
                    ow = work.tile([128, 4, QC], BF, name="ow", tag="ow")
                    for oi in range(4):
                        ot = half * 4 + oi
                        pw = ps_main.tile([128, QC], FP, name="pw", tag="ps")
                        for p in range(2):
                            nc.tensor.matmul(
                                pw[:],
                                wo_sb[:, p, ot * 128:(ot + 1) * 128],
                                att_sb[:, p, qsl],
                                start=(p == 0),
                                stop=(p == 1),
                            )
                        nc.scalar.copy(ow[:, oi, 0:QC // 2], pw[:, 0:QC // 2])
                        nc.vector.tensor_copy(ow[:, oi, QC // 2:QC], pw[:, QC // 2:QC])
                        off = (qi * 8 + half * 4 + oi) * QC
                        out_engs[oi].dma_start(outT[:, off:off + QC], ow[:, oi, :])
                return
            for half in range(2):
                ow = work.tile([128, 4, QC], BF, name="ow", tag="ow")
                for oi in range(4):
                    ot = half * 4 + oi
                    pw = ps_main.tile([128, QC], FP, name="pw", tag="ps")
                    for p in range(2):
                        nc.tensor.matmul(
                            pw[:],
                            wo_sb[:, p, ot * 128:(ot + 1) * 128],
                            att_sb[:, p, qsl],
                            start=(p == 0),
                            stop=(p == 1),
                        )
                    if (oi % 2 == 1) if tailish else (ot % 2 == 1):
                        nc.scalar.copy(ow[:, oi, :], pw[:])
                    else:
                        nc.vector.tensor_copy(ow[:, oi, :], pw[:])
                off = (qi * 2 + half) * 4 * QC
                (nc.gpsimd if half == 0 else nc.sync).dma_start(
                    outT[:, off:off + 4 * QC].rearrange("p (o s) -> p o s", o=4),
                    ow[:],
                )

        # emission: pair0 projections up front with the deferred weight loads
        # slotted behind compute-gated ops so they don't steal SDMA bandwidth
        # from the chunk-0 / wqk gates; then pair1 projections, v, attention
        # and (one chunk behind) o_proj interleaved per q chunk.
        for t in range(NQC):
            if t == 1:
                nc.vector.tensor_copy(cos_sb[64:128, QC:S], cos_sb[0:64, QC:S])
                nc.vector.tensor_copy(ssin_sb[64:128, QC:S], ssin_sb[0:64, QC:S])
            proj_chunk(0, t)
            if t == 0:
                nc.scalar.dma_start(cos_sb[0:64, QC:S], cos2T[:, QC:S])
                nc.scalar.dma_start(ssin_sb[0:64, QC:S], ssin2T[:, QC:S])
                nc.scalar.dma_start(tri_sb[:], maskD.rearrange("p (r c) -> p r c", r=2))
            proj_chunk(2, t)
            if t == 0:
                for h in range(2):
                    nc.scalar.dma_start(
                        wv_sb[:, 4 * h:4 * h + 4, :],
                        wvT[:, h * 1024:(h + 1) * 1024].rearrange("p (k m) -> p k m", k=4),
                    )
            if t == 1:
                for h in range(2):
                    nc.scalar.dma_start(wo_sb[:, h, :], woT[:, h * HID:(h + 1) * HID])
        nc.vector.memset(l_tiles[:], 1.0)
        qi_order = [1, 2, 3, 0]
        loaded = 0
        prev = None
        for qi in qi_order:
            while loaded <= min(qi + 1, NQC - 1):
                proj_chunk(1, loaded)
                proj_chunk(3, loaded)
                for tt in range(4 * loaded, 4 * loaded + 4):
                    v_proj(tt)
                loaded += 1
            l0 = l_tiles[:, 2 * qi, :]
            pa = attention_unit(0, qi, l0)
            rlb0 = division_pre(l0)
            l1 = l_tiles[:, 2 * qi + 1, :]
            pb = attention_unit(1, qi, l1)
            division_post(0, qi, rlb0, *pa)
            rlb1 = division_pre(l1)
            if prev is not None:
                oproj(prev, tailish=(qi == qi_order[-1]))
            division_post(1, qi, rlb1, *pb)
            prev = qi
        oproj(prev, last=True)

        if dbg is not None:
            for rb in range(4):
                nc.sync.dma_start(dbg["dbg_qk"][rb * 128:(rb + 1) * 128, :], qkT[:, rb, :])
            nc.sync.dma_start(dbg["dbg_v"][:], v_sb.rearrange("p t c -> p (t c)"))
            for p in range(2):
                nc.sync.dma_start(dbg["dbg_att"][p * 128:(p + 1) * 128, :], att_sb[:, p, :])


# ---------- host-side shard preparation ----------

def make_core_inputs(hidden_states, cos, sin, w_qkv, w_o):
    """Returns list of 8 in_maps (numpy, bf16 where needed)."""
    bf = ml_dtypes.bfloat16
    hs = np.asarray(hidden_states, np.float32)
    cos = np.asarray(cos, np.float32)
    sin = np.asarray(sin, np.float32)
    w_qkv = np.asarray(w_qkv, np.float32)
    w_o = np.asarray(w_o, np.float32)

    cosT = cos.T
    sinT = sin.T
    cos2T = np.ascontiguousarray(cosT).astype(bf)
    ssin2T = np.ascontiguousarray(sinT).astype(bf)
    # signed rotate-half permutation: out[m] = sign(m) * x[partner(m)]
    # lhsT layout: pmat[k, m] = sign(m) at k = partner(m)
    pmat = np.zeros((128, 128), np.float32)
    for m in range(128):
        d = m % 64
        base = m - d
        if d < 32:
            pmat[base + d + 32, m] = -1.0
        else:
            pmat[base + d - 32, m] = 1.0
    pmat = pmat.astype(bf)

    kp = np.arange(128)[:, None]
    cc = np.arange(128)[None, :]
    tri = (kp <= cc).astype(bf)
    maskD = np.concatenate([tri, tri], axis=1)

    def swz(a):
        # [K*128, M] -> [128, K*M] partition-major (matches SBUF tiles)
        k = a.shape[0] // 128
        return np.ascontiguousarray(
            a.reshape(k, 128, a.shape[1]).transpose(1, 0, 2).reshape(128, -1))

    in_maps = []
    for c in range(8):
        b, g = divmod(c, 4)
        heads = range(4 * g, 4 * g + 4)
        hsT = np.ascontiguousarray(hs[b].T).astype(bf)  # [HID, S]
        # chunk-major swizzle: [128, t, kk, s']
        hs2 = np.ascontiguousarray(
            hsT.reshape(8, 128, 4, 512).transpose(1, 2, 0, 3).reshape(128, -1))
        wq = np.concatenate([w_qkv[h * 64:(h + 1) * 64] for h in heads], 0)
        wk = np.concatenate([w_qkv[HID + h * 64:HID + (h + 1) * 64] for h in heads], 0)
        wv = np.concatenate([w_qkv[2 * HID + h * 64:2 * HID + (h + 1) * 64] for h in heads], 0)
        wqkT = swz(np.ascontiguousarray(np.concatenate([wq, wk], 0).T).astype(bf))
        wvT = swz(np.ascontiguousarray(wv.T).astype(bf))
        woT = swz(np.ascontiguousarray(
            np.concatenate([w_o[:, h * 64:(h + 1) * 64] for h in heads], 1).T
        ).astype(bf))
        in_maps.append({
            "hsT": hs2, "wqkT": wqkT, "wvT": wvT, "woT": woT,
            "cos2T": cos2T, "ssin2T": ssin2T, "maskD": maskD, "pmat": pmat,
        })
    return in_maps


def unswizzle_out(o2):
    # [128, qi*half*oi*512] -> outT [1024, 2048]
    a = o2.reshape(128, NQC, 2, 4, QC)
    return np.ascontiguousarray(
        a.transpose(2, 3, 0, 1, 4).reshape(HID, S))


def unshard(outTs):
    out = np.zeros((B, S, HID), np.float32)
    for c, o2 in enumerate(outTs):
        out[c // 4] += unswizzle_out(o2).T.astype(np.float32)
    return out


# ---------- standalone kernel entry ----------

from concourse.bass_utils import run_bass_kernel_spmd

_CACHED_NC = None


def get_program():
    global _CACHED_NC
    if _CACHED_NC is None:
        _CACHED_NC = build_program()
    return _CACHED_NC


def run(inputs, trace=False):
    nc = get_program()
    in_maps = make_core_inputs(**inputs)
    res = run_bass_kernel_spmd(nc, in_maps, core_ids=list(range(8)), trace=trace)
    out = np.zeros((B, S, HID), np.float32)
    for c, r in enumerate(res.results):
        out[c // 4] += unswizzle_out(r["outT"]).T.astype(np.float32)
    return out, res


def kernel(**inputs):
    out, _ = run(inputs, trace=False)
    return out


# revision 37
# speedup vs baseline: 1.1808x; 1.0137x over previous
"""Sharded causal-attention kernel for 8 trn2 NeuronCores.

DP over batch (2) x TP over head groups (4 heads/core). Each core: qkv projection
(its heads) + RoPE + causal SDPA (scores kept transposed; softmax denominator via a
ones-column in the PV matmul) + its 256-row slice of the o_proj contraction, returning
a transposed partial [HID, S]; the host sums 4 partials per batch. bf16 matmuls,
fp32 PSUM accumulation.

v2: consumption-ordered fine-grained input DMA across the sync/scalar/gpsimd
DGE queues (first matmul gates on ~256KB instead of 1MB, deferred weight loads
slotted behind compute-gated ops); causal-live-column score matmuls; PV delayed
one k-tile behind exp so the PE has independent work while ACT computes exp;
softmax division with fast reciprocal + GpSimd broadcasts hoisted ahead of the
numerator path and small copies split across ACT/DVE; RoPE PSUM->SBUF copy and
rotate matmul split by column halves across ACT/DVE; final o_proj output DMA
fanned out per-128KB tile across queues with copies split across both engines.
"""

import sys

sys.path.insert(0, "/opt/trn_rl_repo")

from contextlib import ExitStack

import numpy as np
import ml_dtypes

import concourse.bass as bass
import concourse.mybir as mybir
import concourse.tile as tile
from concourse import bacc

FP = mybir.dt.float32
BF = mybir.dt.bfloat16
EXP = mybir.ActivationFunctionType.Exp

B, S, HID = 2, 2048, 1024
H, D = 16, 64
QC = 512
KT = 128
NQC = S // QC
NKT = S // KT
KHID = HID // 128


def build_program(debug_outputs=False):
    nc = bacc.Bacc("TRN2", target_bir_lowering=False, debug=False, num_devices=8, num_swdge_queues=4)

    hsT = nc.dram_tensor("hsT", [128, NQC * KHID * QC], BF, kind="ExternalInput").ap()
    wqkT = nc.dram_tensor("wqkT", [128, KHID * 512], BF, kind="ExternalInput").ap()
    wvT = nc.dram_tensor("wvT", [128, KHID * 256], BF, kind="ExternalInput").ap()
    woT = nc.dram_tensor("woT", [128, 2 * HID], BF, kind="ExternalInput").ap()
    cos2T = nc.dram_tensor("cos2T", [64, S], BF, kind="ExternalInput").ap()
    ssin2T = nc.dram_tensor("ssin2T", [64, S], BF, kind="ExternalInput").ap()
    maskD = nc.dram_tensor("maskD", [128, 256], BF, kind="ExternalInput").ap()
    pmat = nc.dram_tensor("pmat", [128, 128], BF, kind="ExternalInput").ap()
    outT = nc.dram_tensor("outT", [128, NQC * 8 * QC], BF, kind="ExternalOutput").ap()
    dbg = None
    if debug_outputs:
        dbg = {
            "dbg_qk": nc.dram_tensor("dbg_qk", [512, S], BF, kind="ExternalOutput").ap(),
            "dbg_v": nc.dram_tensor("dbg_v", [128, NKT * 4 * 65], BF, kind="ExternalOutput").ap(),
            "dbg_att": nc.dram_tensor("dbg_att", [256, S], BF, kind="ExternalOutput").ap(),
        }

    with tile.TileContext(nc) as tc:
        build_tile_program(tc, hsT, wqkT, wvT, woT, cos2T, ssin2T, maskD, pmat, outT, dbg)
    nc.compile()
    return nc


def build_tile_program(tc, hsT, wqkT, wvT, woT, cos2T, ssin2T, maskD, pmat, outT, dbg=None):
    nc = tc.nc
    with ExitStack() as ctx:
        const = ctx.enter_context(tc.tile_pool(name="const", bufs=1))
        persist = ctx.enter_context(tc.tile_pool(name="persist", bufs=1))
        work = ctx.enter_context(tc.tile_pool(name="work", bufs=5))
        posbp = ctx.enter_context(tc.tile_pool(name="posbp", bufs=12))
        expp = ctx.enter_context(tc.tile_pool(name="expp", bufs=8))
        small = ctx.enter_context(tc.tile_pool(name="small", bufs=4))
        ps_main = ctx.enter_context(tc.tile_pool(name="ps_main", bufs=4, space="PSUM"))
        ps_sc = ctx.enter_context(tc.tile_pool(name="ps_sc", bufs=2, space="PSUM"))

        # ---- critical first wave: exactly what the first proj matmuls gate on,
        # in consumption order, spread across all five engine DMA queues so no
        # single queue serializes the gate ----
        wqk_sb = const.tile([128, KHID, 512], BF, name="wqk_sb")
        hs_sb = const.tile([128, NQC, KHID, QC], BF, name="hs_sb")
        cos_sb = const.tile([128, S], BF, name="cos_sb")
        ssin_sb = const.tile([128, S], BF, name="ssin_sb")
        pmat_sb = const.tile([128, 128], BF, name="pmat_sb")
        nc.sync.dma_start(
            wqk_sb[:, 0:2, :],
            wqkT[:, 0:1024].rearrange("p (k m) -> p k m", k=2),
        )
        nc.gpsimd.dma_start(hs_sb[:, 0, 0:1, :], hsT[:, 0:QC].rearrange("p (k s) -> p k s", k=1))
        nc.scalar.dma_start(cos_sb[0:64, 0:QC], cos2T[:, 0:QC])
        nc.sync.dma_start(pmat_sb[:], pmat[:])
        nc.scalar.dma_start(ssin_sb[0:64, 0:QC], ssin2T[:, 0:QC])
        for j, eng in ((1, nc.scalar), (2, nc.sync), (3, nc.scalar)):
            eng.dma_start(
                wqk_sb[:, 2 * j:2 * j + 2, :],
                wqkT[:, j * 1024:(j + 1) * 1024].rearrange("p (k m) -> p k m", k=2),
            )
        for lo_, hi_ in ((1, 3), (3, 5), (5, 8)):
            nc.gpsimd.dma_start(
                hs_sb[:, 0, lo_:hi_, :],
                hsT[:, lo_ * QC:hi_ * QC].rearrange("p (k s) -> p k s", k=hi_ - lo_),
            )
        nc.vector.tensor_copy(cos_sb[64:128, 0:QC], cos_sb[0:64, 0:QC])
        nc.vector.tensor_copy(ssin_sb[64:128, 0:QC], ssin_sb[0:64, 0:QC])
        tri_sb = const.tile([128, 2, 128], BF, name="tri_sb")
        # remaining hs chunks: gpsimd queue, issued behind the chunk-0 pieces
        for t in range(1, NQC):
            for h in range(2):
                nc.gpsimd.dma_start(
                    hs_sb[:, t, 4 * h:4 * h + 4, :],
                    hsT[:, (t * KHID + 4 * h) * QC:(t * KHID + 4 * h + 4) * QC].rearrange(
                        "p (k s) -> p k s", k=4),
                )
        wv_sb = const.tile([128, KHID, 256], BF, name="wv_sb")
        wo_sb = const.tile([128, 2, HID], BF, name="wo_sb")

        qkT = persist.tile([128, 4, S], BF, name="qkT")
        l_tiles = persist.tile([64, 8, QC], FP, name="l_tiles")
        v_sb = persist.tile([128, NKT, 4 * 65], BF, name="v_sb2")
        att_sb = persist.tile([128, 2, S], BF, name="att_sb2")
        nc.vector.memset(
            v_sb.rearrange("p t (h c) -> p t h c", c=65)[:, :, :, 64:65], 1.0
        )

        def proj_chunk(rb, t):
            csl = slice(t * QC, (t + 1) * QC)
            ps = ps_main.tile([128, QC], FP, name="ps_qk", tag="ps")
            for kk in range(KHID):
                nc.tensor.matmul(
                    ps[:],
                    wqk_sb[:, kk, rb * 128:(rb + 1) * 128],
                    hs_sb[:, t, kk, :],
                    start=(kk == 0),
                    stop=(kk == KHID - 1),
                )
            x = work.tile([128, QC], BF, name="x_rope", tag="xrope")
            half = QC // 2
            nc.scalar.copy(x[:, 0:half], ps[:, 0:half])
            nc.vector.tensor_copy(x[:, half:QC], ps[:, half:QC])
            # signed rotate-half on the PE: xs = P @ x (P carries the +-1),
            # split by column half so each starts as soon as its copy lands
            xs_ps = ps_main.tile([128, QC], FP, name="xs_ps", tag="ps")
            nc.tensor.matmul(xs_ps[:, 0:half], pmat_sb[:], x[:, 0:half], start=True, stop=True)
            nc.tensor.matmul(xs_ps[:, half:QC], pmat_sb[:], x[:, half:QC], start=True, stop=True)
            t1 = work.tile([128, QC], BF, name="t1_rope", tag="t1rope")
            t2 = work.tile([128, QC], BF, name="t2_rope", tag="t2rope")
            nc.vector.tensor_mul(t1[:], x[:], cos_sb[:, csl])
            nc.vector.tensor_mul(t2[:], xs_ps[:], ssin_sb[:, csl])
            nc.vector.tensor_add(qkT[:, rb, csl], t1[:], t2[:])

        def v_proj(tt):
            psv = ps_main.tile([128, 256], FP, name="ps_v", tag="ps")
            for kk in range(KHID):
                nc.tensor.matmul(
                    psv[:],
                    hs_sb[:, tt // 4, kk, (tt % 4) * 128:(tt % 4 + 1) * 128],
                    wv_sb[:, kk, :],
                    start=(kk == 0),
                    stop=(kk == KHID - 1),
                )
            nc.scalar.copy(
                v_sb[:, tt, :].rearrange("p (h c) -> p h c", c=65)[:, :, 0:64],
                psv[:].rearrange("p (h c) -> p h c", c=64),
            )

        def attention_unit(qi, l0, l1):
            """Fused scores^T -> exp -> PV for all four heads at q-chunk qi.

            Both head-pairs interleave per k-tile so each exp has ~2x the PE
            cover, and the PV batch trails one k-tile behind. The four PV
            accumulators occupy all four ps_main buffers for the unit.
            Denominators land in l0/l1 rows 0 and 32."""
            qsl = slice(qi * QC, (qi + 1) * QC)
            nki = 4 * qi + 4
            po = [ps_main.tile([65, QC], FP, name=f"po{h}", tag="ps") for h in range(4)]

            def emit_pv(ki, e_a, e_b, lo):
                for h, (e_, sub) in enumerate(((e_a, 0), (e_a, 1), (e_b, 0), (e_b, 1))):
                    nc.tensor.matmul(
                        po[h][:, lo:QC], v_sb[:, ki, h * 65:(h + 1) * 65],
                        e_[:, sub, lo:QC],
                        start=(ki == 0), stop=(ki == nki - 1),
                    )

            pending = []
            for ki in range(nki):
                ksl = slice(ki * KT, (ki + 1) * KT)
                j = ki - 4 * qi
                lo = 0 if j < 0 else 128 * j  # first live q column in this chunk
                es = []
                for pair in range(2):
                    psc = ps_sc.tile([128, 2, QC], FP, name="psc", tag="sc")
                    nc.tensor.matmul(
                        psc[:, 0, lo:QC], qkT[0:64, 2 + pair, ksl],
                        qkT[0:64, pair, qi * QC + lo:(qi + 1) * QC],
                        start=True, stop=True,
                    )
                    nc.tensor.matmul(
                        psc[:, 1, lo:QC], qkT[64:128, 2 + pair, ksl],
                        qkT[64:128, pair, qi * QC + lo:(qi + 1) * QC],
                        start=True, stop=True,
                    )
                    e = expp.tile([128, 2, QC], BF, name="e", tag="exp")
                    nc.scalar.activation(
                        e[:, :, lo:QC], psc[:, :, lo:QC], EXP, scale=0.125
                    )
                    if j >= 0:
                        nc.vector.tensor_mul(
                            e[:, :, lo:lo + 128], e[:, :, lo:lo + 128], tri_sb[:]
                        )
                    es.append(e)
                if pending:
                    emit_pv(*pending.pop(0))
                pending.append((ki, es[0], es[1], lo))
            for p_ in pending:
                emit_pv(*p_)
            # epilogue: denominator rows to l0/l1, numerators to SBUF bf16
            nc.vector.tensor_copy(l0[0:1, :], po[0][64:65, :])
            nc.scalar.copy(l0[32:33, :], po[1][64:65, :])
            nc.vector.tensor_copy(l1[0:1, :], po[2][64:65, :])
            nc.scalar.copy(l1[32:33, :], po[3][64:65, :])
            po_sb = []
            for h in range(4):
                t_ = posbp.tile([64, QC], BF, name=f"po_sb{h}", tag="posb")
                if h % 2 == 0:
                    nc.vector.tensor_copy(t_[:], po[h][0:64, :])
                else:
                    nc.scalar.copy(t_[:], po[h][0:64, :])
                po_sb.append(t_)
            return (po_sb[0], po_sb[1]), (po_sb[2], po_sb[3])

        def division_pre(l_pair):
            """1/l for both heads of a pair, broadcast to 64 partitions.

            Only depends on the denominator rows, so the gpsimd broadcasts
            overlap the next attention unit / o_proj on the PE."""
            rl = small.tile([64, QC], FP, name="rl", tag="rl")
            nc.vector.reciprocal_approx_fast(out=rl[:], in_=l_pair[:])
            rb0_ = small.tile([64, QC], FP, name="rb0_", tag="rbb", bufs=6)
            nc.gpsimd.partition_broadcast(rb0_[:], rl[0:1, :])
            rlrow = small.tile([1, QC], FP, name="rlrow", tag="rlrow", bufs=4)
            nc.scalar.copy(rlrow[:], rl[32:33, :])
            rb1_ = small.tile([64, QC], FP, name="rb1_", tag="rbb", bufs=6)
            nc.gpsimd.partition_broadcast(rb1_[:], rlrow[:])
            return rb0_, rb1_

        def division_post(pair, qi, rb, po_sb0, po_sb1):
            qsl = slice(qi * QC, (qi + 1) * QC)
            rb0_, rb1_ = rb
            nc.vector.tensor_mul(att_sb[0:64, pair, qsl], po_sb0[0:64, :], rb0_[:])
            nc.vector.tensor_mul(att_sb[64:128, pair, qsl], po_sb1[0:64, :], rb1_[:])

        def oproj(qi, last=False, tailish=False):
            qsl = slice(qi * QC, (qi + 1) * QC)
            out_engs = (nc.gpsimd, nc.sync, nc.scalar, nc.gpsimd)
            if last:
                for half in range(2):
                    ow = work.tile([128, 4, QC], BF, name="ow", tag="ow")
                    for oi in range(4):
                        ot = half * 4 + oi
                        pw = ps_main.tile([128, QC], FP, name="pw", tag="ps")
                        for p in range(2):
                            nc.tensor.matmul(
                                pw[:],
                                wo_sb[:, p, ot * 128:(ot + 1) * 128],
                                att_sb[:, p, qsl],
                                start=(p == 0),
                                stop=(p == 1),
                            )
                        nc.scalar.copy(ow[:, oi, 0:QC // 2], pw[:, 0:QC // 2])
                        nc.vector.tensor_copy(ow[:, oi, QC // 2:QC], pw[:, QC // 2:QC])
                        off = (qi * 8 + half * 4 + oi) * QC
                        out_engs[oi].dma_start(outT[:, off:off + QC], ow[:, oi, :])
                return
            for half in range(2):
                ow = work.tile([128, 4, QC], BF, name="ow", tag="ow")
                for oi in range(4):
                    ot = half * 4 + oi
                    pw = ps_main.tile([128, QC], FP, name="pw", tag="ps")
                    for p in range(2):
                        nc.tensor.matmul(
                            pw[:],
                            wo_sb[:, p, ot * 128:(ot + 1) * 128],
                            att_sb[:, p, qsl],
                            start=(p == 0),
                            stop=(p == 1),
                        )
                    if (oi % 2 == 1) if tailish else (ot % 2 == 1):
                        nc.scalar.copy(ow[:, oi, :], pw[:])
                    else:
                        nc.vector.tensor_copy(ow[:, oi, :], pw[:])
                off = (qi * 2 + half) * 4 * QC
                (nc.gpsimd if half == 0 else nc.sync).dma_start(
                    outT[:, off:off + 4 * QC].rearrange("p (o s) -> p o s", o=4),
                    ow[:],
                )

        # emission: pair0 projections up front with the deferred weight loads
        # slotted behind compute-gated ops so they don't steal SDMA bandwidth
        # from the chunk-0 / wqk gates; then pair1 projections, v, attention
        # and (one chunk behind) o_proj interleaved per q chunk.
        for t in range(NQC):
            if t == 1:
                nc.vector.tensor_copy(cos_sb[64:128, QC:S], cos_sb[0:64, QC:S])
                nc.vector.tensor_copy(ssin_sb[64:128, QC:S], ssin_sb[0:64, QC:S])
            proj_chunk(0, t)
            if t == 0:
                nc.scalar.dma_start(cos_sb[0:64, QC:S], cos2T[:, QC:S])
                nc.scalar.dma_start(ssin_sb[0:64, QC:S], ssin2T[:, QC:S])
                nc.scalar.dma_start(tri_sb[:], maskD.rearrange("p (r c) -> p r c", r=2))
            proj_chunk(2, t)
            if t == 0:
                for h in range(2):
                    nc.scalar.dma_start(
                        wv_sb[:, 4 * h:4 * h + 4, :],
                        wvT[:, h * 1024:(h + 1) * 1024].rearrange("p (k m) -> p k m", k=4),
                    )
            if t == 1:
                for h in range(2):
                    nc.scalar.dma_start(wo_sb[:, h, :], woT[:, h * HID:(h + 1) * HID])
        nc.vector.memset(l_tiles[:], 1.0)
        qi_order = [1, 2, 3, 0]
        loaded = 0
        prev = None
        for qi in qi_order:
            while loaded <= min(qi + 1, NQC - 1):
                proj_chunk(1, loaded)
                proj_chunk(3, loaded)
                for tt in range(4 * loaded, 4 * loaded + 4):
                    v_proj(tt)
                loaded += 1
            l0 = l_tiles[:, 2 * qi, :]
            l1 = l_tiles[:, 2 * qi + 1, :]
            pa, pb = attention_unit(qi, l0, l1)
            rlb0 = division_pre(l0)
            rlb1 = division_pre(l1)
            if prev is not None:
                oproj(prev, tailish=(qi == qi_order[-1]))
            division_post(0, qi, rlb0, *pa)
            division_post(1, qi, rlb1, *pb)
            prev = qi
        oproj(prev, last=True)

        if dbg is not None:
            for rb in range(4):
                nc.sync.dma_start(dbg["dbg_qk"][rb * 128:(rb + 1) * 128, :], qkT[:, rb, :])
            nc.sync.dma_start(dbg["dbg_v"][:], v_sb.rearrange("p t c -> p (t c)"))
            for p in range(2):
                nc.sync.dma_start(dbg["dbg_att"][p * 128:(p + 1) * 128, :], att_sb[:, p, :])


# ---------- host-side shard preparation ----------

def make_core_inputs(hidden_states, cos, sin, w_qkv, w_o):
    """Returns list of 8 in_maps (numpy, bf16 where needed)."""
    bf = ml_dtypes.bfloat16
    hs = np.asarray(hidden_states, np.float32)
    cos = np.asarray(cos, np.float32)
    sin = np.asarray(sin, np.float32)
    w_qkv = np.asarray(w_qkv, np.float32)
    w_o = np.asarray(w_o, np.float32)

    cosT = cos.T
    sinT = sin.T
    cos2T = np.ascontiguousarray(cosT).astype(bf)
    ssin2T = np.ascontiguousarray(sinT).astype(bf)
    # signed rotate-half permutation: out[m] = sign(m) * x[partner(m)]
    # lhsT layout: pmat[k, m] = sign(m) at k = partner(m)
    pmat = np.zeros((128, 128), np.float32)
    for m in range(128):
        d = m % 64
        base = m - d
        if d < 32:
            pmat[base + d + 32, m] = -1.0
        else:
            pmat[base + d - 32, m] = 1.0
    pmat = pmat.astype(bf)

    kp = np.arange(128)[:, None]
    cc = np.arange(128)[None, :]
    tri = (kp <= cc).astype(bf)
    maskD = np.concatenate([tri, tri], axis=1)

    def swz(a):
        # [K*128, M] -> [128, K*M] partition-major (matches SBUF tiles)
        k = a.shape[0] // 128
        return np.ascontiguousarray(
            a.reshape(k, 128, a.shape[1]).transpose(1, 0, 2).reshape(128, -1))

    in_maps = []
    for c in range(8):
        b, g = divmod(c, 4)
        heads = range(4 * g, 4 * g + 4)
        hsT = np.ascontiguousarray(hs[b].T).astype(bf)  # [HID, S]
        # chunk-major swizzle: [128, t, kk, s']
        hs2 = np.ascontiguousarray(
            hsT.reshape(8, 128, 4, 512).transpose(1, 2, 0, 3).reshape(128, -1))
        wq = np.concatenate([w_qkv[h * 64:(h + 1) * 64] for h in heads], 0)
        wk = np.concatenate([w_qkv[HID + h * 64:HID + (h + 1) * 64] for h in heads], 0)
        wv = np.concatenate([w_qkv[2 * HID + h * 64:2 * HID + (h + 1) * 64] for h in heads], 0)
        wqkT = swz(np.ascontiguousarray(np.concatenate([wq, wk], 0).T).astype(bf))
        wvT = swz(np.ascontiguousarray(wv.T).astype(bf))
        woT = swz(np.ascontiguousarray(
            np.concatenate([w_o[:, h * 64:(h + 1) * 64] for h in heads], 1).T
        ).astype(bf))
        in_maps.append({
            "hsT": hs2, "wqkT": wqkT, "wvT": wvT, "woT": woT,
            "cos2T": cos2T, "ssin2T": ssin2T, "maskD": maskD, "pmat": pmat,
        })
    return in_maps


def unswizzle_out(o2):
    # [128, qi*half*oi*512] -> outT [1024, 2048]
    a = o2.reshape(128, NQC, 2, 4, QC)
    return np.ascontiguousarray(
        a.transpose(2, 3, 0, 1, 4).reshape(HID, S))


def unshard(outTs):
    out = np.zeros((B, S, HID), np.float32)
    for c, o2 in enumerate(outTs):
        out[c // 4] += unswizzle_out(o2).T.astype(np.float32)
    return out


# ---------- standalone kernel entry ----------

from concourse.bass_utils import run_bass_kernel_spmd

_CACHED_NC = None


def get_program():
    global _CACHED_NC
    if _CACHED_NC is None:
        _CACHED_NC = build_program()
    return _CACHED_NC


def run(inputs, trace=False):
    nc = get_program()
    in_maps = make_core_inputs(**inputs)
    res = run_bass_kernel_spmd(nc, in_maps, core_ids=list(range(8)), trace=trace)
    out = np.zeros((B, S, HID), np.float32)
    for c, r in enumerate(res.results):
        out[c // 4] += unswizzle_out(r["outT"]).T.astype(np.float32)
    return out, res


def kernel(**inputs):
    out, _ = run(inputs, trace=False)
    return out
